# revision 1
# baseline (speedup 1.0000x reference)
"""Trainium2 Bass kernel for nn_CausalSelfAttention (B=1, S=2048, D=1024, H=16).

Tensor-parallel over heads across 8 NeuronCores: core c computes heads
(2c, 2c+1) end-to-end — QKV projection, per-head RMS norm, RoPE, causal
softmax attention, and the row-parallel slice of the output projection.
The host sums the 8 partial outputs (row-parallel Wout) and returns
(y, v1) like the reference.

Device-side layout notes (per core, m = 128 head dims = 2 heads x 64):
  - everything is kept transposed: qT/kT/vT are [m, S] with m on SBUF
    partitions, computed as  W_slice @ x.T  via K-chunked matmuls with
    the host-pretransposed xT as the moving operand.
  - RMS-norm sums run on the PE (indicator matmul over partitions),
    1/sqrt via ACT-Sqrt + DVE reciprocal_approx_fast.  The k-side scale
    rides the Exp activation's per-partition scale AP; the q-side scale
    (with 1/sqrt(hd) folded in) is a broadcast multiply.
  - scores are computed transposed  [k, q]  so softmax-exp output feeds
    the PV matmul directly (no P transposes).  The causal mask of each
    diagonal block is ADDED by the PE (identity @ mask_table matmul into
    the accumulating PSUM group).  Softmax denominators come from a
    ones-column appended to V in the PV stationary operand.
  - all matmuls run as float32r (full-rate FP22 on trn2), fp32 storage.
"""

import os
import sys

import numpy as np

try:
    import concourse.bass as bass  # noqa: F401
except Exception:  # pragma: no cover
    for _p in ("/opt/trn_rl_repo", "/root/.axon_site/_ro/trn_rl_repo"):
        if os.path.isdir(_p) and _p not in sys.path:
            sys.path.insert(0, _p)

import concourse.bacc as bacc
import concourse.bass as bass
import concourse.mybir as mybir
import concourse.tile as tile
from concourse import bass_utils

S = 2048
D = 1024
NH = 16
HD = 64
NCORES = 8
M = (NH // NCORES) * HD  # 128 m-dims per core (2 heads)
NCHUNK = D // 128        # 8 contraction chunks for the projections
NKC = S // 128           # 16 key chunks
HALF = S // 2            # attention processed in q-halves of 1024

F32 = mybir.dt.float32
F32R = mybir.dt.float32r
BF16 = mybir.dt.bfloat16
AF = mybir.ActivationFunctionType
ALU = mybir.AluOpType

EPS = float(np.finfo(np.float32).eps)
NEG = -1e30


def _splits(n0, n1, step=512):
    """[n0, n1) split at absolute multiples of `step`."""
    out = []
    a = n0
    while a < n1:
        b = min(n1, (a // step + 1) * step)
        out.append((a, b))
        a = b
    return out


def r(ap):
    return ap.bitcast(F32R)


def _emit(tc, io):
    nc = tc.nc
    ctx_pools = []

    def pool(*a, **k):
        return tc.alloc_tile_pool(*a, **k)

    consts = pool(name="consts", bufs=1)
    wpool = pool(name="wpool", bufs=1)
    persist = pool(name="persist", bufs=1)
    work = pool(name="work", bufs=2)
    ctx_pools += [consts, wpool, persist, work]

    # ---- constants / weights into SBUF -------------------------------
    ident = consts.tile([128, 128], F32)
    nc.sync.dma_start(out=ident, in_=io["ident"].ap())
    identb = consts.tile([128, 128], BF16)
    nc.sync.dma_start(out=identb, in_=io["identb"].ap())
    maskb = consts.tile([128, 128], BF16)
    nc.sync.dma_start(out=maskb, in_=io["maskb"].ap())
    c4 = consts.tile([4, 2], F32)
    nc.sync.dma_start(out=c4, in_=io["c4"].ap())
    ind8 = consts.tile([128, 8], F32R)
    nc.sync.dma_start(out=ind8, in_=io["ind8"].ap())
    indT2 = consts.tile([2, 128], F32R)
    nc.sync.dma_start(out=indT2, in_=io["indT2"].ap())
    cosT = consts.tile([128, S], F32)
    sinTs = consts.tile([128, S], F32)

    w_sb = {}
    for nm in ("wqT", "wkT", "wvT"):
        t = wpool.tile([128, NCHUNK, 128], F32R, name=nm)
        nc.sync.dma_start(out=t, in_=io[nm].ap().rearrange("(po pi) m -> pi po m", pi=128))
        w_sb[nm] = t
    wo_sb = wpool.tile([128, D], F32R)
    v1_sb = wpool.tile([128, S], F32)

    # ---- persistent activations --------------------------------------
    q_fin = persist.tile([128, S], F32)   # raw qT -> roped+scaled qT in place
    k_fin = persist.tile([128, S], F32)
    vT_sb = persist.tile([128, S], F32)
    v_ext = persist.tile([128, NKC, 132], F32)  # [k, chunk, 66*h..] w/ 2 ones cols per head
    y2T = persist.tile([128, S], F32)

    sq_q = work.tile([128, S], F32, tag="sq")
    sq_k = work.tile([128, S], F32, tag="sq")
    swp_q = work.tile([128, S], F32, tag="swp")
    swp_k = work.tile([128, S], F32, tag="swp")

    # ================= phase A: QKV projections =======================
    xtp = pool(name="xt", bufs=1)
    pqkv = pool(name="pqkv", bufs=2, space="PSUM")
    xt_sb = xtp.tile([128, NCHUNK, S], F32R)
    xt_dram = io["xT"].ap().rearrange("(po pi) s -> pi po s", pi=128)
    for c in range(NCHUNK):
        nc.sync.dma_start(out=xt_sb[:, c, :], in_=xt_dram[:, c, :])
    # loads needed later in the pipeline go after the hot xT chunks
    nc.sync.dma_start(out=cosT, in_=io["cosT"].ap())
    nc.sync.dma_start(out=sinTs, in_=io["sinTs"].ap())
    nc.sync.dma_start(out=v1_sb, in_=io["v1Ts"].ap())
    nc.sync.dma_start(out=wo_sb, in_=io["woT"].ap())

    def proj(wname):
        ps = pqkv.tile([128, S], F32, tag="qkv", name=f"ps_{wname}")
        for c in range(NCHUNK):
            for s0, s1 in _splits(0, S):
                nc.tensor.matmul(
                    ps[:, s0:s1],
                    w_sb[wname][:, c, :],
                    xt_sb[:, c, s0:s1],
                    start=(c == 0),
                    stop=(c == NCHUNK - 1),
                )
        return ps

    ps_q = proj("wqT")
    nc.vector.tensor_copy(out=q_fin.bitcast(F32R), in_=ps_q)           # DVE evac
    nc.gpsimd.tensor_mul(out=sq_q.bitcast(F32R), in0=q_fin, in1=q_fin)
    # rope swap copies (read raw q before it is overwritten in place)
    for d0, s0 in ((0, 32), (32, 0), (64, 96), (96, 64)):
        nc.sync.dma_start(out=swp_q[d0:d0 + 32, :], in_=q_fin[s0:s0 + 32, :])
    nc.vector.tensor_mul(out=q_fin.bitcast(F32R), in0=q_fin, in1=cosT)

    ps_k = proj("wkT")
    nc.scalar.copy(out=k_fin.bitcast(F32R), in_=ps_k)                  # ACT evac
    nc.gpsimd.tensor_mul(out=sq_k.bitcast(F32R), in0=k_fin, in1=k_fin)
    for d0, s0 in ((0, 32), (32, 0), (64, 96), (96, 64)):
        nc.sync.dma_start(out=swp_k[d0:d0 + 32, :], in_=k_fin[s0:s0 + 32, :])
    nc.vector.tensor_mul(out=k_fin.bitcast(F32R), in0=k_fin, in1=cosT)

    ps_v = proj("wvT")
    nc.vector.tensor_add(out=vT_sb, in0=ps_v, in1=v1_sb)  # value-residual mix

    pqkv.release()
    xtp.release()

    # ================= phase B: norms + rope + v transpose ============
    late = pool(name="late", bufs=1)
    pnorm = pool(name="pnorm", bufs=1, space="PSUM")
    pvt = pool(name="pvt", bufs=2, space="PSUM")

    ps_norm = pnorm.tile([4, S], F32, tag="nb")
    for s0, s1 in _splits(0, S):
        nc.tensor.matmul(ps_norm[:, s0:s1], ind8[:, 0:4], r(sq_q[:, s0:s1]),
                         start=True, stop=False)
        nc.tensor.matmul(ps_norm[:, s0:s1], ind8[:, 4:8], r(sq_k[:, s0:s1]),
                         start=False, stop=True)
    sqrt_sb = late.tile([4, S], F32)
    nc.scalar.activation(out=sqrt_sb, in_=ps_norm, func=AF.Sqrt,
                         bias=c4[:, 1:2], scale=c4[:, 0:1])
    inv4 = late.tile([4, S], F32)
    nc.vector.reciprocal_approx_fast(out=inv4, in_=sqrt_sb)

    # q-side scale broadcast (row h -> partitions [64h, 64h+64)) via PE
    invq = late.tile([2, S], F32)
    nc.vector.tensor_copy(out=invq.bitcast(F32R), in_=inv4[0:2, :])
    rq_ps = pnorm.tile([128, S], F32, tag="nb")
    for s0, s1 in _splits(0, S):
        nc.tensor.matmul(rq_ps[:, s0:s1], indT2, r(invq[:, s0:s1]),
                         start=True, stop=True)

    # k-side scale, transposed to [k-position, 4j+2+h]
    ps_invT = pnorm.tile([128, 4 * NKC], F32)
    for t in range(NKC):
        nc.tensor.transpose(ps_invT[:, 4 * t:4 * t + 4],
                            inv4[:, 128 * t:128 * t + 128], ident[0:4, 0:4])
    invT_sb = late.tile([128, 4 * NKC], F32)
    nc.vector.tensor_copy(out=invT_sb, in_=ps_invT)

    # finish rope: x*cos already in q_fin/k_fin; add swap*sin; scale q
    nc.vector.tensor_mul(out=swp_q, in0=swp_q, in1=sinTs)
    nc.vector.tensor_add(out=q_fin.bitcast(F32R), in0=q_fin, in1=swp_q)
    nc.vector.tensor_mul(out=q_fin.bitcast(F32R), in0=q_fin, in1=rq_ps)
    nc.vector.tensor_mul(out=swp_k, in0=swp_k, in1=sinTs)
    nc.vector.tensor_add(out=k_fin.bitcast(F32R), in0=k_fin, in1=swp_k)

    # v: transpose [m, S] -> per-chunk [k, m] with ones columns appended
    ones32 = io["ones16"].ap().rearrange("p (a b) -> p a b", b=2)
    nc.sync.dma_start(out=v_ext[:, :, 64:66].bitcast(F32R), in_=ones32)
    nc.sync.dma_start(out=v_ext[:, :, 130:132].bitcast(F32R), in_=ones32)
    for t in range(NKC):
        ps_vt = pvt.tile([128, 128], F32, tag="vt")
        nc.tensor.transpose(ps_vt, vT_sb[:, 128 * t:128 * t + 128], ident)
        if t % 2 == 0:
            nc.vector.tensor_copy(out=v_ext[:, t, 0:64].bitcast(F32R), in_=ps_vt[:, 0:64])
            nc.scalar.copy(out=v_ext[:, t, 66:130].bitcast(F32R), in_=ps_vt[:, 64:128])
        else:
            nc.scalar.copy(out=v_ext[:, t, 0:64].bitcast(F32R), in_=ps_vt[:, 0:64])
            nc.vector.tensor_copy(out=v_ext[:, t, 66:130].bitcast(F32R), in_=ps_vt[:, 64:128])

    pvt.release()
    pnorm.release()

    # ================= phase C: causal attention ======================
    pattn = pool(name="pattn", bufs=2, space="PSUM")

    for Hf in range(2):
        q0h = HALF * Hf
        yts = [pattn.tile([66, HALF], F32, tag="yt", name=f"yt{Hf}_{h}")
               for h in range(2)]
        njc = 8 * (Hf + 1)
        for j in range(njc):
            exs = []
            pss = []
            q0 = max(q0h, 128 * j)
            qn = q0h + HALF - q0
            diag = (q0 == 128 * j)
            for h in range(2):
                ps_s = pattn.tile([128, HALF], F32, tag="sc", name=f"sc{Hf}_{j}_{h}")
                pss.append(ps_s)
                for idx, (l0, l1) in enumerate(_splits(0, qn)):
                    nc.tensor.matmul(
                        ps_s[:, l0:l1],
                        r(k_fin[64 * h:64 * h + 64, 128 * j:128 * j + 128]),
                        r(q_fin[64 * h:64 * h + 64, q0 + l0:q0 + l1]),
                        start=True, stop=not (diag and idx == 0),
                    )
            for h in range(2):
                if diag:  # leading block is on the diagonal: add causal bias
                    nc.tensor.matmul(pss[h][:, 0:128], identb, maskb,
                                     start=False, stop=True)
                ex = late.tile([128, HALF], F32, tag="ex", bufs=3,
                               name=f"ex{Hf}_{j}_{h}")
                nc.scalar.activation(out=ex[:, :qn].bitcast(F32R), in_=pss[h][:, :qn], func=AF.Exp,
                                     bias=0.0,
                                     scale=invT_sb[:, 4 * j + 2 + h:4 * j + 3 + h])
                exs.append((ex, q0, qn))
            for h in range(2):
                ex, q0, qn = exs[h]
                ystart = q0 - q0h
                nxt = max(0, 128 * (j + 1) - q0h)  # next strip's local start
                for w0, w1 in _splits(ystart, HALF):
                    nc.tensor.matmul(
                        yts[h][:, w0:w1],
                        r(v_ext[:, j, 66 * h:66 * h + 66]),
                        r(ex[:, w0 - ystart:w1 - ystart]),
                        start=(j == 0),
                        stop=(j == njc - 1) or (w1 <= nxt),
                    )
        for h in range(2):
            if "dbg_yraw" in io and Hf == 0 and h == 0:
                dbg_yr_sb = late.tile([66, HALF], F32, name="dbg_yr_sb")
                nc.vector.tensor_copy(out=dbg_yr_sb, in_=yts[h])
                nc.sync.dma_start(out=io["dbg_yraw"].ap(), in_=dbg_yr_sb)
            den = late.tile([2, HALF], F32, tag="den", bufs=2,
                            name=f"den{Hf}_{h}")
            nc.vector.tensor_copy(out=den, in_=yts[h][64:66, :])
            scr = late.tile([2, HALF], F32, tag="scr", bufs=2,
                            name=f"scr{Hf}_{h}")
            nc.vector.reciprocal_approx_fast(out=scr, in_=den)
            inv2 = late.tile([2, HALF], F32, tag="ivr", bufs=2,
                             name=f"ivr{Hf}_{h}")
            nc.vector.tensor_copy(out=inv2.bitcast(F32R), in_=scr)
            inv_ps = pattn.tile([64, HALF], F32, tag="sc", name=f"ivp{Hf}_{h}")
            for l0, l1 in _splits(0, HALF):
                nc.tensor.matmul(inv_ps[:, l0:l1], indT2[0:2, 0:64],
                                 r(inv2[:, l0:l1]), start=True, stop=True)
            inv_b = late.tile([64, HALF], F32, tag="ivb", bufs=2,
                              name=f"ivb{Hf}_{h}")
            nc.vector.tensor_copy(out=inv_b, in_=inv_ps)
            if "dbg_invb" in io and Hf == 0 and h == 0:
                nc.sync.dma_start(out=io["dbg_invb"].ap(), in_=inv_b)
            nc.vector.tensor_mul(out=y2T[64 * h:64 * h + 64, q0h:q0h + HALF].bitcast(F32R),
                                 in0=yts[h][0:64, :], in1=inv_b)

    pattn.release()

    # ================= phase D: output projection =====================
    pout = pool(name="pout", bufs=2, space="PSUM")
    outp = io["outp"].ap()
    for oc in range(8):
        ps_o = pout.tile([128, S], F32, tag="o", name=f"o{oc}")
        for s0, s1 in _splits(0, S):
            nc.tensor.matmul(ps_o[:, s0:s1],
                             wo_sb[:, 128 * oc:128 * oc + 128],
                             r(y2T[:, s0:s1]), start=True, stop=True)
        osb = late.tile([128, S], F32, tag="osb", bufs=2, name=f"osb{oc}")
        if oc % 2 == 0:
            nc.vector.tensor_copy(out=osb, in_=ps_o)
        else:
            nc.scalar.copy(out=osb, in_=ps_o)
        nc.sync.dma_start(out=outp[oc], in_=osb)
    pout.release()

    if "dbg_q" in io:
        nc.sync.dma_start(out=io["dbg_q"].ap(), in_=q_fin)
        nc.sync.dma_start(out=io["dbg_k"].ap(), in_=k_fin)
        nc.sync.dma_start(out=io["dbg_vext"].ap(), in_=v_ext)
        nc.sync.dma_start(out=io["dbg_inv4"].ap(), in_=inv4)
        nc.sync.dma_start(out=io["dbg_invT"].ap(), in_=invT_sb)
        nc.sync.dma_start(out=io["dbg_y2T"].ap(), in_=y2T)

    late.release()
    for p in reversed(ctx_pools):
        p.release()


_CACHE = {}


def _build(debug_taps=False):
    key = ("nc", debug_taps)
    if key in _CACHE:
        return _CACHE[key]
    nc = bacc.Bacc("TRN2", target_bir_lowering=False, debug=False,
                   enable_asserts=True, num_devices=NCORES)
    io = {}
    io["xT"] = nc.dram_tensor("xT", [D, S], F32R, kind="ExternalInput")
    io["cosT"] = nc.dram_tensor("cosT", [128, S], F32, kind="ExternalInput")
    io["sinTs"] = nc.dram_tensor("sinTs", [128, S], F32, kind="ExternalInput")
    io["ident"] = nc.dram_tensor("ident", [128, 128], F32, kind="ExternalInput")
    io["identb"] = nc.dram_tensor("identb", [128, 128], BF16, kind="ExternalInput")
    io["maskb"] = nc.dram_tensor("maskb", [128, 128], BF16, kind="ExternalInput")
    io["c4"] = nc.dram_tensor("c4", [4, 2], F32, kind="ExternalInput")
    io["ind8"] = nc.dram_tensor("ind8", [128, 8], F32R, kind="ExternalInput")
    io["indT2"] = nc.dram_tensor("indT2", [2, 128], F32R, kind="ExternalInput")
    io["ones16"] = nc.dram_tensor("ones16", [128, 32], F32R, kind="ExternalInput")
    io["wqT"] = nc.dram_tensor("wqT", [D, M], F32R, kind="ExternalInput")
    io["wkT"] = nc.dram_tensor("wkT", [D, M], F32R, kind="ExternalInput")
    io["wvT"] = nc.dram_tensor("wvT", [D, M], F32R, kind="ExternalInput")
    io["woT"] = nc.dram_tensor("woT", [M, D], F32R, kind="ExternalInput")
    io["v1Ts"] = nc.dram_tensor("v1Ts", [M, S], F32, kind="ExternalInput")
    io["outp"] = nc.dram_tensor("outp", [8, 128, S], F32, kind="ExternalOutput")
    if debug_taps:
        io["dbg_q"] = nc.dram_tensor("dbg_q", [128, S], F32, kind="ExternalOutput")
        io["dbg_k"] = nc.dram_tensor("dbg_k", [128, S], F32, kind="ExternalOutput")
        io["dbg_vext"] = nc.dram_tensor("dbg_vext", [128, NKC, 132], F32, kind="ExternalOutput")
        io["dbg_inv4"] = nc.dram_tensor("dbg_inv4", [4, S], F32, kind="ExternalOutput")
        io["dbg_invT"] = nc.dram_tensor("dbg_invT", [128, 4 * NKC], F32, kind="ExternalOutput")
        io["dbg_y2T"] = nc.dram_tensor("dbg_y2T", [128, S], F32, kind="ExternalOutput")
        io["dbg_yraw"] = nc.dram_tensor("dbg_yraw", [66, HALF], F32, kind="ExternalOutput")
        io["dbg_invb"] = nc.dram_tensor("dbg_invb", [64, HALF], F32, kind="ExternalOutput")

    with tile.TileContext(nc) as tc:
        _emit(tc, io)
    nc.compile()
    _CACHE[key] = nc
    return nc


def _host_prep(x, v1, Wq, Wk, Wv, Wout, lambdas):
    """Build per-core input maps (all fp32 numpy)."""
    x = np.asarray(x, np.float32).reshape(S, D)
    v1 = np.asarray(v1, np.float32).reshape(S, D)
    Wq = np.asarray(Wq, np.float32)
    Wk = np.asarray(Wk, np.float32)
    Wv = np.asarray(Wv, np.float32)
    Wout = np.asarray(Wout, np.float32)
    lam = np.float32(np.asarray(lambdas))

    xT = np.ascontiguousarray(x.T)

    inv_freq = (np.float32(1.0)
                / np.power(np.float32(10000.0),
                           np.arange(0, HD, 2, dtype=np.float32) / np.float32(HD)))
    t = np.arange(S, dtype=np.float32)
    freqs = np.outer(t, inv_freq).astype(np.float32)       # [S, 32]
    cos = np.cos(freqs).T                                   # [32, S]
    sin = np.sin(freqs).T
    cosT = np.ascontiguousarray(np.tile(cos, (4, 1)), dtype=np.float32)
    sinTs = np.ascontiguousarray(
        np.concatenate([sin, -sin, sin, -sin], axis=0), dtype=np.float32)

    ident = np.eye(128, dtype=np.float32)
    import ml_dtypes
    identb = np.eye(128, dtype=ml_dtypes.bfloat16)
    kk, qq = np.meshgrid(np.arange(128), np.arange(128), indexing="ij")
    maskb = np.where(qq >= kk, 0.0, NEG).astype(ml_dtypes.bfloat16)

    c4 = np.array([[1.0, 64.0 * EPS],
                   [1.0, 64.0 * EPS],
                   [1.0 / 64.0, EPS],
                   [1.0 / 64.0, EPS]], dtype=np.float32)
    # norms rows: 0-1 q (h0, h1), 2-3 k (h0, h1); q matmul uses cols 0:4,
    # k matmul uses cols 4:8 (slice-local column index = psum row)
    ind8 = np.zeros((128, 8), dtype=np.float32)
    ind8[0:64, 0] = 1.0     # q h0 -> norms row 0
    ind8[64:128, 1] = 1.0   # q h1 -> norms row 1
    ind8[0:64, 6] = 1.0     # k h0 -> norms row 2
    ind8[64:128, 7] = 1.0   # k h1 -> norms row 3

    indT2 = np.zeros((2, 128), dtype=np.float32)
    indT2[0, 0:64] = 1.0
    indT2[1, 64:128] = 1.0

    ones16 = np.ones((128, 32), dtype=np.float32)

    shared = dict(xT=xT, cosT=cosT, sinTs=sinTs, ident=ident, identb=identb,
                  maskb=maskb, c4=c4, ind8=ind8, indT2=indT2, ones16=ones16)

    in_maps = []
    for c in range(NCORES):
        sl = slice(128 * c, 128 * c + 128)
        m = dict(shared)
        m["wqT"] = np.ascontiguousarray(Wq[sl, :].T)
        m["wkT"] = np.ascontiguousarray(Wk[sl, :].T)
        m["wvT"] = np.ascontiguousarray(((np.float32(1.0) - lam) * Wv[sl, :]).T)
        m["woT"] = np.ascontiguousarray(Wout[:, sl].T)
        m["v1Ts"] = np.ascontiguousarray((lam * v1[:, sl]).T)
        in_maps.append(m)
    return in_maps


def run(inputs, trace=False):
    nh = int(np.asarray(inputs["n_heads"]))
    assert nh == NH, f"kernel compiled for n_heads={NH}, got {nh}"
    nc = _build()
    in_maps = _host_prep(inputs["x"], inputs["v1"], inputs["Wq"], inputs["Wk"],
                         inputs["Wv"], inputs["Wout"], inputs["lambdas"])
    res = bass_utils.run_bass_kernel_spmd(
        nc, in_maps, core_ids=list(range(NCORES)), trace=trace)
    outT = np.zeros((D, S), dtype=np.float32)
    for c in range(NCORES):
        outT += res.results[c]["outp"].reshape(D, S)
    y = np.ascontiguousarray(outT.T).reshape(1, S, D).astype(np.float32)
    v1 = np.asarray(inputs["v1"], np.float32).reshape(1, S, D)
    return (y, v1), res


def kernel(**inputs):
    outs, _ = run(inputs, trace=False)
    return outs

